# revision 11
# baseline (speedup 1.0000x reference)
"""GQA attention block (RMSNorm-QK + RoPE + causal attention + proj) on 8 TRN2 cores.

Sharding: DP=2 over batch x TP=4 over heads (4 q heads + 1 kv head per core).
Per core: x[b] @ Wq_shard / Wkv_shard -> q,k,v; RMSNorm+RoPE (cos/sin tables
precomputed on host, q_scale/k_scale and 1/sqrt(HS) baked in); causal
flash-ish attention in bf16 with f32 softmax stats; AllGather of y^T over the
4 TP ranks; column-parallel Wproj. Host pre-transposes x so the device never
transposes activations for the projections.
"""

import math
import os
import sys

import numpy as np

for _p in ("/opt/trn_rl_repo", "/root/.axon_site/_ro/trn_rl_repo"):
    if os.path.isdir(_p) and _p not in sys.path:
        sys.path.insert(0, _p)

import ml_dtypes

import concourse.bacc as bacc
import concourse.mybir as mybir
import concourse.tile as tile
from concourse import masks
from concourse.bass_utils import run_bass_kernel_spmd

BF16 = mybir.dt.bfloat16
F32 = mybir.dt.float32
AX = mybir.AxisListType
ALU = mybir.AluOpType
AF = mybir.ActivationFunctionType

B, T, C = 2, 2048, 2048
NH, NKV, HS = 16, 4, 128
TP = 4                # tensor-parallel ranks per batch element
QH = NH // TP         # q heads per core
QW = QH * HS          # 512
PT = 128
NT = T // PT          # 16
NCT = C // PT         # 16
H2 = HS // 2
EPS = 1e-6
THETA = 10000.0
NCORES = 8
BF = ml_dtypes.bfloat16

_CACHE = {}


def _build(loop_r=None):
    nc = bacc.Bacc(None, target_bir_lowering=False, num_devices=NCORES)

    xT = nc.declare_dram_parameter("xT", [C, T], BF16, isOutput=False)
    wq = nc.declare_dram_parameter("wq", [C, QW], BF16, isOutput=False)
    wkv = nc.declare_dram_parameter("wkv", [C, 2 * HS], BF16, isOutput=False)
    wp = nc.declare_dram_parameter("wp", [C, QW], BF16, isOutput=False)
    v1s = nc.declare_dram_parameter("v1s", [T, HS], F32, isOutput=False)
    cosq = nc.declare_dram_parameter("cosq", [T, QW], BF16, isOutput=False)
    sinq = nc.declare_dram_parameter("sinq", [T, QW], BF16, isOutput=False)
    cosk = nc.declare_dram_parameter("cosk", [T, HS], BF16, isOutput=False)
    sink = nc.declare_dram_parameter("sink", [T, HS], BF16, isOutput=False)
    mneg = nc.declare_dram_parameter("mneg", [PT, PT], F32, isOutput=False)
    out = nc.declare_dram_parameter("out", [T, QW], F32, isOutput=True)

    groups = [[0, 1, 2, 3], [4, 5, 6, 7]]

    with tile.TileContext(nc) as tc:
        with (
            tc.tile_pool(name="const", bufs=1) as const,
            tc.tile_pool(name="persist", bufs=1) as persist,
            tc.tile_pool(name="psum", bufs=1, space="PSUM") as psum,
            tc.tile_pool(name="wk", bufs=3) as wk,
            tc.tile_pool(name="dram", bufs=1, space="DRAM") as dram,
        ):
            ident = const.tile([PT, PT], BF16)
            masks.make_identity(nc, ident[:])
            maskt = const.tile([PT, PT], F32)
            nc.sync.dma_start(maskt[:], mneg[:])
            eps_t = const.tile([PT, 1], F32)
            nc.gpsimd.memset(eps_t[:], EPS)
            ones_t = const.tile([PT, 1], BF16)
            nc.gpsimd.memset(ones_t[:], 1.0)

            wq_s = persist.tile([PT, NCT, QW], BF16)
            nc.sync.dma_start(wq_s[:], wq[:].rearrange("(c p) m -> p c m", p=PT))
            wkv_s = persist.tile([PT, NCT, 2 * HS], BF16)
            nc.sync.dma_start(wkv_s[:], wkv[:].rearrange("(c p) m -> p c m", p=PT))
            wp_s = persist.tile([PT, NCT, QW], BF16)
            nc.sync.dma_start(wp_s[:], wp[:].rearrange("(c p) m -> p c m", p=PT))

            qT_s = persist.tile([PT, QH, T], BF16)
            kT_s = persist.tile([PT, T], BF16)
            v_s = persist.tile([PT, NT, HS], BF16)
            yT_s = persist.tile([PT, QH, T], BF16)

            ag_in = dram.tile([QW, T], BF16)
            ag_out = dram.tile([C, T], BF16)

            def _load_xt(xt_s):
                nc.sync.dma_start(xt_s[:], xT[:].rearrange("(c p) t -> p c t", p=PT))

            def _stage23(xt_s):
                for ti in range(NT):
                    t0 = ti * PT
                    # ---- QKV projection (lhsT = xT tile, rhs = weight) ----
                    qp = psum.tile([PT, QW], F32, tag="qp", bufs=1)
                    for ci in range(NCT):
                        nc.tensor.matmul(
                            qp[:], xt_s[:, ci, t0:t0 + PT], wq_s[:, ci, :],
                            start=(ci == 0), stop=(ci == NCT - 1),
                        )
                    kvp = psum.tile([PT, 2 * HS], F32, tag="kvp", bufs=1)
                    for ci in range(NCT):
                        nc.tensor.matmul(
                            kvp[:], xt_s[:, ci, t0:t0 + PT], wkv_s[:, ci, :],
                            start=(ci == 0), stop=(ci == NCT - 1),
                        )

                    # ---- RMSNorm stats ----
                    sq = wk.tile([PT, QW], F32, tag="sq")
                    nc.scalar.square(sq[:], qp[:])
                    ms = wk.tile([PT, QH], F32, tag="ms")
                    nc.vector.tensor_reduce(
                        ms[:], sq[:].rearrange("p (h d) -> p h d", d=HS), AX.X, ALU.add
                    )
                    sqk = wk.tile([PT, HS], F32, tag="sqk")
                    nc.scalar.square(sqk[:], kvp[:, 0:HS])
                    msk = wk.tile([PT, 1], F32, tag="msk")
                    nc.vector.tensor_reduce(msk[:], sqk[:], AX.X, ALU.add)

                    rs = wk.tile([PT, QH], F32, tag="rs")
                    nc.scalar.activation(rs[:], ms[:], AF.Sqrt, bias=eps_t[:], scale=1.0 / HS)
                    nc.vector.reciprocal(rs[:], rs[:])
                    rsk = wk.tile([PT, 1], F32, tag="rsk")
                    nc.scalar.activation(rsk[:], msk[:], AF.Sqrt, bias=eps_t[:], scale=1.0 / HS)
                    nc.vector.reciprocal(rsk[:], rsk[:])

                    # ---- normalize (per-head scalar) ----
                    qn = wk.tile([PT, QH, HS], BF16, tag="qn")
                    for h in range(QH):
                        nc.vector.tensor_scalar_mul(
                            qn[:, h, :], qp[:, h * HS:(h + 1) * HS], rs[:, h:h + 1]
                        )
                    kn = wk.tile([PT, HS], BF16, tag="kn")
                    nc.vector.tensor_scalar_mul(kn[:], kvp[:, 0:HS], rsk[:])

                    # ---- RoPE: out = z*cos + rot(z)*sin, tables baked with scales ----
                    qrot = wk.tile([PT, QH, HS], BF16, tag="qrot")
                    nc.vector.tensor_scalar_mul(qrot[:, :, 0:H2], qn[:, :, H2:HS], -1.0)
                    nc.vector.tensor_copy(qrot[:, :, H2:HS], qn[:, :, 0:H2])
                    krot = wk.tile([PT, HS], BF16, tag="krot")
                    nc.vector.tensor_scalar_mul(krot[:, 0:H2], kn[:, H2:HS], -1.0)
                    nc.vector.tensor_copy(krot[:, H2:HS], kn[:, 0:H2])

                    cqt = wk.tile([PT, QW], BF16, tag="cqt")
                    nc.sync.dma_start(cqt[:], cosq[t0:t0 + PT, :])
                    sqt = wk.tile([PT, QW], BF16, tag="sqt")
                    nc.sync.dma_start(sqt[:], sinq[t0:t0 + PT, :])
                    ckt = wk.tile([PT, HS], BF16, tag="ckt")
                    nc.sync.dma_start(ckt[:], cosk[t0:t0 + PT, :])
                    skt = wk.tile([PT, HS], BF16, tag="skt")
                    nc.sync.dma_start(skt[:], sink[t0:t0 + PT, :])

                    qn2 = qn[:].rearrange("p h d -> p (h d)")
                    qrot2 = qrot[:].rearrange("p h d -> p (h d)")
                    qr = wk.tile([PT, QW], BF16, tag="qr")
                    nc.vector.tensor_tensor(qr[:], qn2, cqt[:], ALU.mult)
                    nc.vector.tensor_tensor(qrot2, qrot2, sqt[:], ALU.mult)
                    nc.vector.tensor_tensor(qr[:], qr[:], qrot2, ALU.add)

                    kr = wk.tile([PT, HS], BF16, tag="kr")
                    nc.vector.tensor_tensor(kr[:], kn[:], ckt[:], ALU.mult)
                    nc.vector.tensor_tensor(krot[:], krot[:], skt[:], ALU.mult)
                    nc.vector.tensor_tensor(kr[:], kr[:], krot[:], ALU.add)

                    # ---- v mix ----
                    v1t = wk.tile([PT, HS], F32, tag="v1t")
                    nc.sync.dma_start(v1t[:], v1s[t0:t0 + PT, :])
                    nc.vector.tensor_tensor(v_s[:, ti, :], kvp[:, HS:2 * HS], v1t[:], ALU.add)

                    # ---- transposes q,k -> qT, kT ----
                    qr3 = qr[:].rearrange("p (h d) -> p h d", d=HS)
                    for h in range(QH):
                        tq = psum.tile([PT, PT], BF16, tag="spT", bufs=2)
                        nc.tensor.transpose(tq[:], qr3[:, h, :], ident[:])
                        nc.vector.tensor_copy(qT_s[:, h, t0:t0 + PT], tq[:])
                    tk = psum.tile([PT, PT], BF16, tag="spT", bufs=2)
                    nc.tensor.transpose(tk[:], kr[:], ident[:])
                    nc.vector.tensor_copy(kT_s[:, t0:t0 + PT], tk[:])

                    # ---- causal attention row ti (transposed-scores scheme) ----
                    # s^T[t2,t1] = k^T(stationary) . q^T(moving); exp gives p^T
                    # directly in SBUF; row sums via ones-vector matmuls into the
                    # same PSUM bank as y; normalize fused into the y copy.
                    nk = ti + 1
                    for h in range(QH):
                        yp = psum.tile([PT, HS], F32, tag="yp", bufs=2)
                        rsum = psum.tile([PT, 1], F32, tag="rsum", bufs=2)
                        for j in range(nk):
                            c0 = j * PT
                            spT = psum.tile([PT, PT], F32, tag="spT", bufs=2)
                            nc.tensor.matmul(
                                spT[:], kT_s[:, c0:c0 + PT], qT_s[:, h, t0:t0 + PT],
                                start=True, stop=True,
                            )
                            if j == ti:
                                nc.vector.tensor_tensor(
                                    spT[:], spT[:], maskt[:], ALU.add
                                )
                            pts = wk.tile([PT, PT], BF16, tag="pts", bufs=3)
                            nc.scalar.activation(pts[:], spT[:], AF.Exp)
                            nc.tensor.matmul(
                                yp[:], pts[:], v_s[:, j, :],
                                start=(j == 0), stop=(j == nk - 1),
                            )
                            nc.tensor.matmul(
                                rsum[:], pts[:], ones_t[:],
                                start=(j == 0), stop=(j == nk - 1),
                            )
                        rinv = wk.tile([PT, 1], F32, tag="rinv", bufs=2)
                        nc.vector.reciprocal(rinv[:], rsum[:])
                        y_sb = wk.tile([PT, HS], BF16, tag="y_sb", bufs=2)
                        nc.vector.tensor_scalar_mul(y_sb[:], yp[:], rinv[:])
                        ty = psum.tile([PT, PT], BF16, tag="spT", bufs=2)
                        nc.tensor.transpose(ty[:], y_sb[:], ident[:])
                        nc.vector.tensor_copy(yT_s[:, h, t0:t0 + PT], ty[:])

            def _proj(ytf):
                # ---- output projection (column shard) ----
                for ti in range(NT):
                    t0 = ti * PT
                    pp = psum.tile([PT, QW], F32, tag="qp", bufs=1)
                    for ci in range(NCT):
                        nc.tensor.matmul(
                            pp[:], ytf[:, ci, t0:t0 + PT], wp_s[:, ci, :],
                            start=(ci == 0), stop=(ci == NCT - 1),
                        )
                    ot = wk.tile([PT, QW], F32, tag="ot", bufs=2)
                    nc.vector.tensor_copy(ot[:], pp[:])
                    nc.sync.dma_start(out[t0:t0 + PT, :], ot[:])

            if loop_r is None:
                with tc.tile_pool(name="xtp", bufs=1) as xtp:
                    xt_s = xtp.tile([PT, NCT, T], BF16)
                    _load_xt(xt_s)
                    _stage23(xt_s)
                # ---- AllGather y^T over the TP group ----
                nc.sync.dma_start(ag_in[:].rearrange("(h p) t -> p h t", p=PT), yT_s[:])
                nc.gpsimd.collective_compute(
                    "AllGather", ALU.bypass, replica_groups=groups,
                    ins=[ag_in[:]], outs=[ag_out[:]],
                )
                with tc.tile_pool(name="ytfp", bufs=1) as ytfp:
                    ytf = ytfp.tile([PT, NCT, T], BF16)
                    nc.sync.dma_start(ytf[:], ag_out[:].rearrange("(c p) t -> p c t", p=PT))
                    _proj(ytf)
            else:
                # timing-only build: loop the whole compute body on-device;
                # proj consumes xt_s (same shape as gathered y^T) - numerics
                # are wrong but per-iteration work matches the real kernel
                # minus the AllGather.
                with tc.tile_pool(name="xtp", bufs=1) as xtp:
                    xt_s = xtp.tile([PT, NCT, T], BF16)
                    with tc.For_i(0, loop_r, 1):
                        _load_xt(xt_s)
                        _stage23(xt_s)
                        _proj(xt_s)

    nc.compile()
    return nc


def _tables(q_scale, k_scale):
    inv_freq = THETA ** (-np.arange(0, HS, 2, dtype=np.float64) / HS)
    ang = np.arange(T, dtype=np.float64)[:, None] * inv_freq[None, :]
    cosw = np.concatenate([np.cos(ang), np.cos(ang)], 1)  # (T, 128)
    sinw = np.concatenate([np.sin(ang), np.sin(ang)], 1)
    qs = np.asarray(q_scale, np.float64)
    ks = np.asarray(k_scale, np.float64)
    qs_rot = np.concatenate([qs[H2:], qs[:H2]])
    ks_rot = np.concatenate([ks[H2:], ks[:H2]])
    s = 1.0 / math.sqrt(HS)
    cosq = np.tile((cosw * qs[None, :] * s).astype(BF), (1, QH))
    sinq = np.tile((sinw * qs_rot[None, :] * s).astype(BF), (1, QH))
    cosk = (cosw * ks[None, :]).astype(BF)
    sink = (sinw * ks_rot[None, :]).astype(BF)
    return cosq, sinq, cosk, sink


def _make_in_maps(x, Wq, Wkv, Wproj, q_scale, k_scale, v1, value_lambda, layer_idx):
    x = np.asarray(x, np.float32)
    Wq = np.asarray(Wq, np.float32)
    Wkv = np.asarray(Wkv, np.float32)
    Wproj = np.asarray(Wproj, np.float32)

    li = int(np.asarray(layer_idx))
    mix = (v1 is not None) and (value_lambda is not None) and li > 0
    lam = float(np.asarray(value_lambda).reshape(())) if mix else 1.0

    cosq, sinq, cosk, sink = _tables(q_scale, k_scale)
    mneg = (np.tril(np.ones((PT, PT), np.float32), k=-1) * -1e30).astype(np.float32)

    in_maps = []
    for core in range(NCORES):
        b, r = core // TP, core % TP
        kcols = Wkv[:, r * HS:(r + 1) * HS]
        vcols = Wkv[:, NKV * HS + r * HS: NKV * HS + (r + 1) * HS]
        if mix:
            v1s_np = ((1.0 - lam) * np.asarray(v1, np.float32)[b, :, r, :]).astype(np.float32)
        else:
            v1s_np = np.zeros((T, HS), np.float32)
        in_maps.append({
            "xT": np.ascontiguousarray(x[b].T).astype(BF),
            "wq": Wq[:, r * QW:(r + 1) * QW].astype(BF),
            "wkv": np.ascontiguousarray(np.concatenate([kcols, vcols], 1)).astype(BF),
            "wp": np.ascontiguousarray(Wproj[:, r * QW:(r + 1) * QW]).astype(BF),
            "v1s": v1s_np,
            "cosq": cosq, "sinq": sinq, "cosk": cosk, "sink": sink,
            "mneg": mneg,
        })
    return in_maps


def kernel(x, Wq, Wkv, Wproj, q_scale, k_scale, v1, value_lambda, layer_idx):
    in_maps = _make_in_maps(x, Wq, Wkv, Wproj, q_scale, k_scale, v1,
                            value_lambda, layer_idx)
    if "nc" not in _CACHE:
        _CACHE["nc"] = _build()
    nc = _CACHE["nc"]

    trace = bool(int(os.environ.get("BASS_KERNEL_TRACE", "0")))
    res = run_bass_kernel_spmd(nc, in_maps, core_ids=list(range(NCORES)), trace=trace)
    _CACHE["last"] = res

    y = np.empty((B, T, C), np.float32)
    for core in range(NCORES):
        b, r = core // TP, core % TP
        y[b, :, r * QW:(r + 1) * QW] = np.asarray(res.results[core]["out"])
    return y


# revision 12
# speedup vs baseline: 1.2975x; 1.2975x over previous
"""GQA attention block (RMSNorm-QK + RoPE + causal attention + proj) on 8 TRN2 cores.

Sharding: DP=2 over batch x TP=4 over heads (4 q heads + 1 kv head per core).
Per core: x[b] @ Wq_shard / Wkv_shard -> q,k,v; RMSNorm+RoPE (cos/sin tables
precomputed on host, q_scale/k_scale and 1/sqrt(HS) baked in); causal
flash-ish attention in bf16 with f32 softmax stats; AllGather of y^T over the
4 TP ranks; column-parallel Wproj. Host pre-transposes x so the device never
transposes activations for the projections.
"""

import math
import os
import sys

import numpy as np

for _p in ("/opt/trn_rl_repo", "/root/.axon_site/_ro/trn_rl_repo"):
    if os.path.isdir(_p) and _p not in sys.path:
        sys.path.insert(0, _p)

import ml_dtypes

import concourse.bacc as bacc
import concourse.mybir as mybir
import concourse.tile as tile
from concourse import masks
from concourse.bass_utils import run_bass_kernel_spmd

BF16 = mybir.dt.bfloat16
F32 = mybir.dt.float32
AX = mybir.AxisListType
ALU = mybir.AluOpType
AF = mybir.ActivationFunctionType

B, T, C = 2, 2048, 2048
NH, NKV, HS = 16, 4, 128
TP = 4                # tensor-parallel ranks per batch element
QH = NH // TP         # q heads per core
QW = QH * HS          # 512
PT = 128
NT = T // PT          # 16
NCT = C // PT         # 16
H2 = HS // 2
EPS = 1e-6
THETA = 10000.0
NCORES = 8
BF = ml_dtypes.bfloat16

_CACHE = {}


def _build(loop_r=None):
    nc = bacc.Bacc(None, target_bir_lowering=False, num_devices=NCORES)

    xT = nc.declare_dram_parameter("xT", [C, T], BF16, isOutput=False)
    wq = nc.declare_dram_parameter("wq", [C, QW], BF16, isOutput=False)
    wkv = nc.declare_dram_parameter("wkv", [C, 2 * HS], BF16, isOutput=False)
    wp = nc.declare_dram_parameter("wp", [C, QW], BF16, isOutput=False)
    v1s = nc.declare_dram_parameter("v1s", [T, HS], F32, isOutput=False)
    cosq = nc.declare_dram_parameter("cosq", [T, QW], BF16, isOutput=False)
    sinq = nc.declare_dram_parameter("sinq", [T, QW], BF16, isOutput=False)
    cosk = nc.declare_dram_parameter("cosk", [T, HS], BF16, isOutput=False)
    sink = nc.declare_dram_parameter("sink", [T, HS], BF16, isOutput=False)
    mneg = nc.declare_dram_parameter("mneg", [PT, PT], F32, isOutput=False)
    out = nc.declare_dram_parameter("out", [T, QW], F32, isOutput=True)

    groups = [[0, 1, 2, 3], [4, 5, 6, 7]]

    with tile.TileContext(nc) as tc:
        with (
            tc.tile_pool(name="const", bufs=1) as const,
            tc.tile_pool(name="persist", bufs=1) as persist,
            tc.tile_pool(name="psum", bufs=1, space="PSUM") as psum,
            tc.tile_pool(name="wk", bufs=3) as wk,
            tc.tile_pool(name="dram", bufs=1, space="DRAM") as dram,
        ):
            ident = const.tile([PT, PT], BF16)
            masks.make_identity(nc, ident[:])
            maskt = const.tile([PT, PT], F32)
            nc.sync.dma_start(maskt[:], mneg[:])
            eps_t = const.tile([PT, 1], F32)
            nc.gpsimd.memset(eps_t[:], EPS)
            ones_t = const.tile([PT, 1], BF16)
            nc.gpsimd.memset(ones_t[:], 1.0)

            wq_s = persist.tile([PT, NCT, QW], BF16)
            nc.sync.dma_start(wq_s[:], wq[:].rearrange("(c p) m -> p c m", p=PT))
            wkv_s = persist.tile([PT, NCT, 2 * HS], BF16)
            nc.sync.dma_start(wkv_s[:], wkv[:].rearrange("(c p) m -> p c m", p=PT))
            wp_s = persist.tile([PT, NCT, QW], BF16)
            nc.sync.dma_start(wp_s[:], wp[:].rearrange("(c p) m -> p c m", p=PT))

            qT_s = persist.tile([PT, QH, T], BF16)
            kT_s = persist.tile([PT, T], BF16)
            v_s = persist.tile([PT, NT, HS + 1], BF16)
            nc.gpsimd.memset(v_s[:, :, HS:HS + 1], 1.0)
            yT_s = persist.tile([PT, QH, T], BF16)
            q_all = persist.tile([PT, NT, QW], BF16)
            k_all = persist.tile([PT, NT, HS], BF16)
            ms_all = persist.tile([PT, NT, QH], F32)
            msk_all = persist.tile([PT, NT], F32)
            rs_all = persist.tile([PT, NT, QH], F32)
            rsk_all = persist.tile([PT, NT], F32)

            ag_in = dram.tile([QW, T], BF16)
            ag_out = dram.tile([C, T], BF16)

            def _load_xt(xt_s):
                nc.sync.dma_start(xt_s[:], xT[:].rearrange("(c p) t -> p c t", p=PT))

            def _stage23(xt_s):
                # Phase A: QKV projections + moment stats (ACT: Square only)
                for ti in range(NT):
                    t0 = ti * PT
                    qp = psum.tile([PT, QW], F32, tag="a", bufs=2)
                    for ci in range(NCT):
                        nc.tensor.matmul(
                            qp[:], xt_s[:, ci, t0:t0 + PT], wq_s[:, ci, :],
                            start=(ci == 0), stop=(ci == NCT - 1),
                        )
                    kvp = psum.tile([PT, 2 * HS], F32, tag="b", bufs=2)
                    for ci in range(NCT):
                        nc.tensor.matmul(
                            kvp[:], xt_s[:, ci, t0:t0 + PT], wkv_s[:, ci, :],
                            start=(ci == 0), stop=(ci == NCT - 1),
                        )
                    sq = wk.tile([PT, QW], F32, tag="sq")
                    nc.scalar.square(sq[:], qp[:])
                    nc.vector.tensor_reduce(
                        ms_all[:, ti, :], sq[:].rearrange("p (h d) -> p h d", d=HS),
                        AX.X, ALU.add,
                    )
                    sqk = wk.tile([PT, HS], F32, tag="sqk")
                    nc.scalar.square(sqk[:], kvp[:, 0:HS])
                    nc.vector.tensor_reduce(msk_all[:, ti:ti + 1], sqk[:], AX.X, ALU.add)
                    nc.vector.tensor_copy(q_all[:, ti, :], qp[:])
                    nc.vector.tensor_copy(k_all[:, ti, :], kvp[:, 0:HS])
                    v1t = wk.tile([PT, HS], F32, tag="v1t")
                    nc.sync.dma_start(v1t[:], v1s[t0:t0 + PT, :])
                    nc.vector.tensor_tensor(
                        v_s[:, ti, 0:HS], kvp[:, HS:2 * HS], v1t[:], ALU.add
                    )

                # Phase A2: batched rsqrt (ACT: Sqrt once)
                rs_f = rs_all[:].rearrange("p n h -> p (n h)")
                ms_f = ms_all[:].rearrange("p n h -> p (n h)")
                nc.scalar.activation(rs_f, ms_f, AF.Sqrt, bias=eps_t[:], scale=1.0 / HS)
                nc.vector.reciprocal(rs_f, rs_f)
                nc.scalar.activation(rsk_all[:], msk_all[:], AF.Sqrt, bias=eps_t[:], scale=1.0 / HS)
                nc.vector.reciprocal(rsk_all[:], rsk_all[:])

                # Phase B: normalize + RoPE + q/k transposes (no ACT)
                for ti in range(NT):
                    t0 = ti * PT
                    qn = wk.tile([PT, QH, HS], BF16, tag="qn")
                    for h in range(QH):
                        nc.vector.tensor_scalar_mul(
                            qn[:, h, :], q_all[:, ti, h * HS:(h + 1) * HS],
                            rs_all[:, ti, h:h + 1],
                        )
                    kn = wk.tile([PT, HS], BF16, tag="kn")
                    nc.vector.tensor_scalar_mul(kn[:], k_all[:, ti, :], rsk_all[:, ti:ti + 1])

                    qrot = wk.tile([PT, QH, HS], BF16, tag="qrot")
                    nc.vector.tensor_scalar_mul(qrot[:, :, 0:H2], qn[:, :, H2:HS], -1.0)
                    nc.vector.tensor_copy(qrot[:, :, H2:HS], qn[:, :, 0:H2])
                    krot = wk.tile([PT, HS], BF16, tag="krot")
                    nc.vector.tensor_scalar_mul(krot[:, 0:H2], kn[:, H2:HS], -1.0)
                    nc.vector.tensor_copy(krot[:, H2:HS], kn[:, 0:H2])

                    cqt = wk.tile([PT, QW], BF16, tag="cqt")
                    nc.sync.dma_start(cqt[:], cosq[t0:t0 + PT, :])
                    sqt = wk.tile([PT, QW], BF16, tag="sqt")
                    nc.sync.dma_start(sqt[:], sinq[t0:t0 + PT, :])
                    ckt = wk.tile([PT, HS], BF16, tag="ckt")
                    nc.sync.dma_start(ckt[:], cosk[t0:t0 + PT, :])
                    skt = wk.tile([PT, HS], BF16, tag="skt")
                    nc.sync.dma_start(skt[:], sink[t0:t0 + PT, :])

                    qn2 = qn[:].rearrange("p h d -> p (h d)")
                    qrot2 = qrot[:].rearrange("p h d -> p (h d)")
                    qr = wk.tile([PT, QW], BF16, tag="qr")
                    nc.vector.tensor_tensor(qr[:], qn2, cqt[:], ALU.mult)
                    nc.vector.tensor_tensor(qrot2, qrot2, sqt[:], ALU.mult)
                    nc.vector.tensor_tensor(qr[:], qr[:], qrot2, ALU.add)

                    kr = wk.tile([PT, HS], BF16, tag="kr")
                    nc.vector.tensor_tensor(kr[:], kn[:], ckt[:], ALU.mult)
                    nc.vector.tensor_tensor(krot[:], krot[:], skt[:], ALU.mult)
                    nc.vector.tensor_tensor(kr[:], kr[:], krot[:], ALU.add)

                    qr3 = qr[:].rearrange("p (h d) -> p h d", d=HS)
                    for h in range(QH):
                        tq = psum.tile([PT, PT], BF16, tag="c", bufs=2)
                        nc.tensor.transpose(tq[:], qr3[:, h, :], ident[:])
                        nc.vector.tensor_copy(qT_s[:, h, t0:t0 + PT], tq[:])
                    tk = psum.tile([PT, PT], BF16, tag="d", bufs=2)
                    nc.tensor.transpose(tk[:], kr[:], ident[:])
                    nc.vector.tensor_copy(kT_s[:, t0:t0 + PT], tk[:])

                # Phase C: causal attention, row pairs (ACT: Exp only).
                # s^T = k^T(stat) . q^T(mov, 256 wide); exp -> p^T in SBUF;
                # y+rowsum from one matmul vs v_aug (ones column).
                for mi in range(NT // 2):
                    tA, tB = 2 * mi, 2 * mi + 1
                    a0, b0 = tA * PT, tB * PT
                    for h in range(QH):
                        ypA = psum.tile([PT, HS + 1], F32, tag="c", bufs=2)
                        ypB = psum.tile([PT, HS + 1], F32, tag="d", bufs=2)
                        for j in range(tA + 1):
                            spT = psum.tile([PT, 2 * PT], F32, tag="a", bufs=2)
                            nc.tensor.matmul(
                                spT[:], kT_s[:, j * PT:(j + 1) * PT],
                                qT_s[:, h, a0:a0 + 2 * PT], start=True, stop=True,
                            )
                            if j == tA:
                                nc.vector.tensor_tensor(
                                    spT[:, 0:PT], spT[:, 0:PT], maskt[:], ALU.add
                                )
                            pts = wk.tile([PT, 2 * PT], BF16, tag="pts", bufs=3)
                            nc.scalar.activation(pts[:], spT[:], AF.Exp)
                            nc.tensor.matmul(
                                ypA[:], pts[:, 0:PT], v_s[:, j, :],
                                start=(j == 0), stop=(j == tA),
                            )
                            nc.tensor.matmul(
                                ypB[:], pts[:, PT:2 * PT], v_s[:, j, :],
                                start=(j == 0), stop=False,
                            )
                        spTb = psum.tile([PT, PT], F32, tag="b", bufs=2)
                        nc.tensor.matmul(
                            spTb[:], kT_s[:, b0:b0 + PT], qT_s[:, h, b0:b0 + PT],
                            start=True, stop=True,
                        )
                        nc.vector.tensor_tensor(spTb[:], spTb[:], maskt[:], ALU.add)
                        ptsb = wk.tile([PT, PT], BF16, tag="pts", bufs=3)
                        nc.scalar.activation(ptsb[:], spTb[:], AF.Exp)
                        nc.tensor.matmul(
                            ypB[:], ptsb[:], v_s[:, tB, :], start=False, stop=True,
                        )
                        for x0, ypX in ((a0, ypA), (b0, ypB)):
                            rinv = wk.tile([PT, 1], F32, tag="rinv", bufs=2)
                            nc.vector.reciprocal(rinv[:], ypX[:, HS:HS + 1])
                            y_sb = wk.tile([PT, HS], BF16, tag="y_sb", bufs=2)
                            nc.vector.tensor_scalar_mul(y_sb[:], ypX[:, 0:HS], rinv[:])
                            ty = psum.tile([PT, PT], BF16, tag="b", bufs=2)
                            nc.tensor.transpose(ty[:], y_sb[:], ident[:])
                            nc.vector.tensor_copy(yT_s[:, h, x0:x0 + PT], ty[:])

            def _proj(ytf):
                # ---- output projection (column shard) ----
                for ti in range(NT):
                    t0 = ti * PT
                    pp = psum.tile([PT, QW], F32, tag="a", bufs=2)
                    for ci in range(NCT):
                        nc.tensor.matmul(
                            pp[:], ytf[:, ci, t0:t0 + PT], wp_s[:, ci, :],
                            start=(ci == 0), stop=(ci == NCT - 1),
                        )
                    ot = wk.tile([PT, QW], F32, tag="ot", bufs=2)
                    nc.vector.tensor_copy(ot[:], pp[:])
                    nc.sync.dma_start(out[t0:t0 + PT, :], ot[:])

            if loop_r is None:
                with tc.tile_pool(name="xtp", bufs=1) as xtp:
                    xt_s = xtp.tile([PT, NCT, T], BF16)
                    _load_xt(xt_s)
                    _stage23(xt_s)
                # ---- AllGather y^T over the TP group ----
                nc.sync.dma_start(ag_in[:].rearrange("(h p) t -> p h t", p=PT), yT_s[:])
                nc.gpsimd.collective_compute(
                    "AllGather", ALU.bypass, replica_groups=groups,
                    ins=[ag_in[:]], outs=[ag_out[:]],
                )
                with tc.tile_pool(name="ytfp", bufs=1) as ytfp:
                    ytf = ytfp.tile([PT, NCT, T], BF16)
                    nc.sync.dma_start(ytf[:], ag_out[:].rearrange("(c p) t -> p c t", p=PT))
                    _proj(ytf)
            else:
                # timing-only build: loop the whole compute body on-device;
                # proj consumes xt_s (same shape as gathered y^T) - numerics
                # are wrong but per-iteration work matches the real kernel
                # minus the AllGather.
                with tc.tile_pool(name="xtp", bufs=1) as xtp:
                    xt_s = xtp.tile([PT, NCT, T], BF16)
                    with tc.For_i(0, loop_r, 1):
                        _load_xt(xt_s)
                        _stage23(xt_s)
                        _proj(xt_s)

    nc.compile()
    return nc


def _tables(q_scale, k_scale):
    inv_freq = THETA ** (-np.arange(0, HS, 2, dtype=np.float64) / HS)
    ang = np.arange(T, dtype=np.float64)[:, None] * inv_freq[None, :]
    cosw = np.concatenate([np.cos(ang), np.cos(ang)], 1)  # (T, 128)
    sinw = np.concatenate([np.sin(ang), np.sin(ang)], 1)
    qs = np.asarray(q_scale, np.float64)
    ks = np.asarray(k_scale, np.float64)
    qs_rot = np.concatenate([qs[H2:], qs[:H2]])
    ks_rot = np.concatenate([ks[H2:], ks[:H2]])
    s = 1.0 / math.sqrt(HS)
    cosq = np.tile((cosw * qs[None, :] * s).astype(BF), (1, QH))
    sinq = np.tile((sinw * qs_rot[None, :] * s).astype(BF), (1, QH))
    cosk = (cosw * ks[None, :]).astype(BF)
    sink = (sinw * ks_rot[None, :]).astype(BF)
    return cosq, sinq, cosk, sink


def _make_in_maps(x, Wq, Wkv, Wproj, q_scale, k_scale, v1, value_lambda, layer_idx):
    x = np.asarray(x, np.float32)
    Wq = np.asarray(Wq, np.float32)
    Wkv = np.asarray(Wkv, np.float32)
    Wproj = np.asarray(Wproj, np.float32)

    li = int(np.asarray(layer_idx))
    mix = (v1 is not None) and (value_lambda is not None) and li > 0
    lam = float(np.asarray(value_lambda).reshape(())) if mix else 1.0

    cosq, sinq, cosk, sink = _tables(q_scale, k_scale)
    mneg = (np.tril(np.ones((PT, PT), np.float32), k=-1) * -1e30).astype(np.float32)

    in_maps = []
    for core in range(NCORES):
        b, r = core // TP, core % TP
        kcols = Wkv[:, r * HS:(r + 1) * HS]
        vcols = Wkv[:, NKV * HS + r * HS: NKV * HS + (r + 1) * HS]
        if mix:
            v1s_np = ((1.0 - lam) * np.asarray(v1, np.float32)[b, :, r, :]).astype(np.float32)
        else:
            v1s_np = np.zeros((T, HS), np.float32)
        in_maps.append({
            "xT": np.ascontiguousarray(x[b].T).astype(BF),
            "wq": Wq[:, r * QW:(r + 1) * QW].astype(BF),
            "wkv": np.ascontiguousarray(np.concatenate([kcols, vcols], 1)).astype(BF),
            "wp": np.ascontiguousarray(Wproj[:, r * QW:(r + 1) * QW]).astype(BF),
            "v1s": v1s_np,
            "cosq": cosq, "sinq": sinq, "cosk": cosk, "sink": sink,
            "mneg": mneg,
        })
    return in_maps


def kernel(x, Wq, Wkv, Wproj, q_scale, k_scale, v1, value_lambda, layer_idx):
    in_maps = _make_in_maps(x, Wq, Wkv, Wproj, q_scale, k_scale, v1,
                            value_lambda, layer_idx)
    if "nc" not in _CACHE:
        _CACHE["nc"] = _build()
    nc = _CACHE["nc"]

    trace = bool(int(os.environ.get("BASS_KERNEL_TRACE", "0")))
    res = run_bass_kernel_spmd(nc, in_maps, core_ids=list(range(NCORES)), trace=trace)
    _CACHE["last"] = res

    y = np.empty((B, T, C), np.float32)
    for core in range(NCORES):
        b, r = core // TP, core % TP
        y[b, :, r * QW:(r + 1) * QW] = np.asarray(res.results[core]["out"])
    return y


# revision 16
# speedup vs baseline: 1.3460x; 1.0374x over previous
"""GQA attention block (RMSNorm-QK + RoPE + causal attention + proj) on 8 TRN2 cores.

Sharding: DP=2 over batch x TP=4 over heads (4 q heads + 1 kv head per core).
Per core: x[b] @ Wq_shard / Wkv_shard -> q,k,v; RMSNorm+RoPE (cos/sin tables
precomputed on host, q_scale/k_scale and 1/sqrt(HS) baked in); causal
flash-ish attention in bf16 with f32 softmax stats; AllGather of y^T over the
4 TP ranks; column-parallel Wproj. Host pre-transposes x so the device never
transposes activations for the projections.
"""

import math
import os
import sys

import numpy as np

for _p in ("/opt/trn_rl_repo", "/root/.axon_site/_ro/trn_rl_repo"):
    if os.path.isdir(_p) and _p not in sys.path:
        sys.path.insert(0, _p)

import ml_dtypes

import concourse.bacc as bacc
import concourse.mybir as mybir
import concourse.tile as tile
from concourse import masks
from concourse.bass_utils import run_bass_kernel_spmd

BF16 = mybir.dt.bfloat16
F32 = mybir.dt.float32
AX = mybir.AxisListType
ALU = mybir.AluOpType
AF = mybir.ActivationFunctionType

B, T, C = 2, 2048, 2048
NH, NKV, HS = 16, 4, 128
TP = 4                # tensor-parallel ranks per batch element
QH = NH // TP         # q heads per core
QW = QH * HS          # 512
PT = 128
NT = T // PT          # 16
NCT = C // PT         # 16
H2 = HS // 2
EPS = 1e-6
THETA = 10000.0
NCORES = 8
BF = ml_dtypes.bfloat16

_CACHE = {}


def _build(loop_r=None):
    nc = bacc.Bacc(None, target_bir_lowering=False, num_devices=NCORES)

    xT = nc.declare_dram_parameter("xT", [C, T], BF16, isOutput=False)
    wq = nc.declare_dram_parameter("wq", [C, QW], BF16, isOutput=False)
    wkv = nc.declare_dram_parameter("wkv", [C, 2 * HS], BF16, isOutput=False)
    wp = nc.declare_dram_parameter("wp", [C, QW], BF16, isOutput=False)
    v1s = nc.declare_dram_parameter("v1s", [T, HS], F32, isOutput=False)
    cosq = nc.declare_dram_parameter("cosq", [T, QW], BF16, isOutput=False)
    sinq = nc.declare_dram_parameter("sinq", [T, QW], BF16, isOutput=False)
    cosk = nc.declare_dram_parameter("cosk", [T, HS], BF16, isOutput=False)
    sink = nc.declare_dram_parameter("sink", [T, HS], BF16, isOutput=False)
    mneg = nc.declare_dram_parameter("mneg", [PT, PT], F32, isOutput=False)
    out = nc.declare_dram_parameter("out", [T, QW], F32, isOutput=True)

    groups = [[0, 1, 2, 3], [4, 5, 6, 7]]

    with tile.TileContext(nc) as tc:
        with (
            tc.tile_pool(name="const", bufs=1) as const,
            tc.tile_pool(name="persist", bufs=1) as persist,
            tc.tile_pool(name="psum", bufs=1, space="PSUM") as psum,
            tc.tile_pool(name="wk", bufs=3) as wk,
            tc.tile_pool(name="dram", bufs=1, space="DRAM") as dram,
        ):
            ident = const.tile([PT, PT], BF16)
            masks.make_identity(nc, ident[:])
            maskt = const.tile([PT, PT], F32)
            nc.sync.dma_start(maskt[:], mneg[:])
            eps_t = const.tile([PT, 1], F32)
            nc.gpsimd.memset(eps_t[:], EPS)
            ones_t = const.tile([PT, 1], BF16)
            nc.gpsimd.memset(ones_t[:], 1.0)

            wq_s = persist.tile([PT, NCT, QW], BF16)
            wkv_s = persist.tile([PT, NCT, 2 * HS], BF16)
            wp_s = persist.tile([PT, NCT, QW], BF16)
            for ci in range(NCT):
                nc.sync.dma_start(wq_s[:, ci, :], wq[ci * PT:(ci + 1) * PT, :])
                nc.sync.dma_start(wkv_s[:, ci, :], wkv[ci * PT:(ci + 1) * PT, :])
                nc.sync.dma_start(wp_s[:, ci, :], wp[ci * PT:(ci + 1) * PT, :])

            qT_s = persist.tile([PT, QH, T], BF16)
            kT_s = persist.tile([PT, T], BF16)
            v_s = persist.tile([PT, NT, HS + 1], BF16)
            nc.gpsimd.memset(v_s[:, :, HS:HS + 1], 1.0)
            yT_s = persist.tile([PT, QH, T], BF16)
            q_all = persist.tile([PT, NT, QW], BF16)
            k_all = persist.tile([PT, NT, HS], BF16)
            ms_all = persist.tile([PT, NT, QH], F32)
            msk_all = persist.tile([PT, NT], F32)
            rs_all = persist.tile([PT, NT, QH], F32)
            rsk_all = persist.tile([PT, NT], F32)

            TC = 256
            NCH = T // TC
            ag_ins = [dram.tile([QW, TC], BF16, name=f"ag_in{c}") for c in range(NCH)]
            ag_outs = [dram.tile([C, TC], BF16, name=f"ag_out{c}") for c in range(NCH)]

            def _load_xt(xt_s):
                for ci in range(NCT):
                    nc.sync.dma_start(xt_s[:, ci, :], xT[ci * PT:(ci + 1) * PT, :])

            def _ag_issue(c):
                # gather chunk c of y^T across the TP group
                c0 = c * TC
                nc.sync.dma_start(
                    ag_ins[c][:].rearrange("(h p) t -> p h t", p=PT),
                    yT_s[:, :, c0:c0 + TC],
                )
                nc.gpsimd.collective_compute(
                    "AllGather", ALU.bypass, replica_groups=groups,
                    ins=[ag_ins[c][:]], outs=[ag_outs[c][:]],
                )

            def _proj_chunk(c, src, local):
                # project t-chunk c: 4 row-tiles of 128
                for tt in range(TC // PT):
                    ti = c * (TC // PT) + tt
                    lt0 = (tt if local else ti) * PT
                    pp = psum.tile([PT, QW], F32, tag="e", bufs=1)
                    for ci in range(NCT):
                        nc.tensor.matmul(
                            pp[:], src[:, ci, lt0:lt0 + PT], wp_s[:, ci, :],
                            start=(ci == 0), stop=(ci == NCT - 1),
                        )
                    ot = wk.tile([PT, QW], F32, tag="ot", bufs=2)
                    nc.vector.tensor_copy(ot[:], pp[:])
                    nc.sync.dma_start(out[ti * PT:(ti + 1) * PT, :], ot[:])

            def _stage23(xt_s, proj_cb=None):
                # Phase A: QKV projections + moment stats (ACT: Square only)
                for ti in range(NT):
                    t0 = ti * PT
                    qp = psum.tile([PT, QW], F32, tag="a", bufs=2)
                    for ci in range(NCT):
                        nc.tensor.matmul(
                            qp[:], xt_s[:, ci, t0:t0 + PT], wq_s[:, ci, :],
                            start=(ci == 0), stop=(ci == NCT - 1),
                        )
                    kvp = psum.tile([PT, 2 * HS], F32, tag="b", bufs=2)
                    for ci in range(NCT):
                        nc.tensor.matmul(
                            kvp[:], xt_s[:, ci, t0:t0 + PT], wkv_s[:, ci, :],
                            start=(ci == 0), stop=(ci == NCT - 1),
                        )
                    sq = wk.tile([PT, QW], F32, tag="sq", bufs=2)
                    nc.scalar.square(sq[:], qp[:])
                    nc.vector.tensor_reduce(
                        ms_all[:, ti, :], sq[:].rearrange("p (h d) -> p h d", d=HS),
                        AX.X, ALU.add,
                    )
                    sqk = wk.tile([PT, HS], F32, tag="sqk", bufs=2)
                    nc.scalar.square(sqk[:], kvp[:, 0:HS])
                    nc.vector.tensor_reduce(msk_all[:, ti:ti + 1], sqk[:], AX.X, ALU.add)
                    nc.vector.tensor_copy(q_all[:, ti, :], qp[:])
                    nc.vector.tensor_copy(k_all[:, ti, :], kvp[:, 0:HS])
                    v1t = wk.tile([PT, HS], F32, tag="v1t", bufs=2)
                    nc.sync.dma_start(v1t[:], v1s[t0:t0 + PT, :])
                    nc.vector.tensor_tensor(
                        v_s[:, ti, 0:HS], kvp[:, HS:2 * HS], v1t[:], ALU.add
                    )

                # Phase A2: batched rsqrt (ACT: Sqrt once)
                rs_f = rs_all[:].rearrange("p n h -> p (n h)")
                ms_f = ms_all[:].rearrange("p n h -> p (n h)")
                nc.scalar.activation(rs_f, ms_f, AF.Sqrt, bias=eps_t[:], scale=1.0 / HS)
                nc.vector.reciprocal(rs_f, rs_f)
                nc.scalar.activation(rsk_all[:], msk_all[:], AF.Sqrt, bias=eps_t[:], scale=1.0 / HS)
                nc.vector.reciprocal(rsk_all[:], rsk_all[:])

                # Phase B: normalize + RoPE + q/k transposes (no ACT)
                for ti in range(NT):
                    t0 = ti * PT
                    qn = wk.tile([PT, QH, HS], BF16, tag="qn", bufs=2)
                    for h in range(QH):
                        nc.vector.tensor_scalar_mul(
                            qn[:, h, :], q_all[:, ti, h * HS:(h + 1) * HS],
                            rs_all[:, ti, h:h + 1],
                        )
                    kn = wk.tile([PT, HS], BF16, tag="kn")
                    nc.vector.tensor_scalar_mul(kn[:], k_all[:, ti, :], rsk_all[:, ti:ti + 1])

                    qrot = wk.tile([PT, QH, HS], BF16, tag="qrot", bufs=2)
                    nc.vector.tensor_scalar_mul(qrot[:, :, 0:H2], qn[:, :, H2:HS], -1.0)
                    nc.vector.tensor_copy(qrot[:, :, H2:HS], qn[:, :, 0:H2])
                    krot = wk.tile([PT, HS], BF16, tag="krot")
                    nc.vector.tensor_scalar_mul(krot[:, 0:H2], kn[:, H2:HS], -1.0)
                    nc.vector.tensor_copy(krot[:, H2:HS], kn[:, 0:H2])

                    cqt = wk.tile([PT, QW], BF16, tag="cqt", bufs=2)
                    nc.sync.dma_start(cqt[:], cosq[t0:t0 + PT, :])
                    sqt = wk.tile([PT, QW], BF16, tag="sqt", bufs=2)
                    nc.sync.dma_start(sqt[:], sinq[t0:t0 + PT, :])
                    ckt = wk.tile([PT, HS], BF16, tag="ckt")
                    nc.sync.dma_start(ckt[:], cosk[t0:t0 + PT, :])
                    skt = wk.tile([PT, HS], BF16, tag="skt")
                    nc.sync.dma_start(skt[:], sink[t0:t0 + PT, :])

                    qn2 = qn[:].rearrange("p h d -> p (h d)")
                    qrot2 = qrot[:].rearrange("p h d -> p (h d)")
                    qr = wk.tile([PT, QW], BF16, tag="qr", bufs=2)
                    nc.vector.tensor_tensor(qr[:], qn2, cqt[:], ALU.mult)
                    nc.vector.tensor_tensor(qrot2, qrot2, sqt[:], ALU.mult)
                    nc.vector.tensor_tensor(qr[:], qr[:], qrot2, ALU.add)

                    kr = wk.tile([PT, HS], BF16, tag="kr")
                    nc.vector.tensor_tensor(kr[:], kn[:], ckt[:], ALU.mult)
                    nc.vector.tensor_tensor(krot[:], krot[:], skt[:], ALU.mult)
                    nc.vector.tensor_tensor(kr[:], kr[:], krot[:], ALU.add)

                    qr3 = qr[:].rearrange("p (h d) -> p h d", d=HS)
                    for h in range(QH):
                        tq = psum.tile([PT, PT], BF16, tag="c", bufs=2)
                        nc.tensor.transpose(tq[:], qr3[:, h, :], ident[:])
                        nc.vector.tensor_copy(qT_s[:, h, t0:t0 + PT], tq[:])
                    tk = psum.tile([PT, PT], BF16, tag="c", bufs=2)
                    nc.tensor.transpose(tk[:], kr[:], ident[:])
                    nc.vector.tensor_copy(kT_s[:, t0:t0 + PT], tk[:])

                # Phase C: causal attention, row pairs (ACT: Exp only).
                # s^T = k^T(stat) . q^T(mov, 256 wide); exp -> p^T in SBUF;
                # y+rowsum from one matmul vs v_aug (ones column).
                for mi in range(NT // 2):
                    tA, tB = 2 * mi, 2 * mi + 1
                    a0, b0 = tA * PT, tB * PT
                    for h in range(QH):
                        ypA = psum.tile([PT, HS + 1], F32, tag="c", bufs=2)
                        ypB = psum.tile([PT, HS + 1], F32, tag="d", bufs=1)
                        for j in range(tA + 1):
                            spT = psum.tile([PT, 2 * PT], F32, tag="a", bufs=2)
                            nc.tensor.matmul(
                                spT[:], kT_s[:, j * PT:(j + 1) * PT],
                                qT_s[:, h, a0:a0 + 2 * PT], start=True, stop=True,
                            )
                            if j == tA:
                                nc.vector.tensor_tensor(
                                    spT[:, 0:PT], spT[:, 0:PT], maskt[:], ALU.add
                                )
                            pts = wk.tile([PT, 2 * PT], BF16, tag="pts", bufs=3)
                            nc.scalar.activation(pts[:], spT[:], AF.Exp)
                            nc.tensor.matmul(
                                ypA[:], pts[:, 0:PT], v_s[:, j, :],
                                start=(j == 0), stop=(j == tA),
                            )
                            nc.tensor.matmul(
                                ypB[:], pts[:, PT:2 * PT], v_s[:, j, :],
                                start=(j == 0), stop=False,
                            )
                        spTb = psum.tile([PT, PT], F32, tag="a", bufs=2)
                        nc.tensor.matmul(
                            spTb[:], kT_s[:, b0:b0 + PT], qT_s[:, h, b0:b0 + PT],
                            start=True, stop=True,
                        )
                        nc.vector.tensor_tensor(spTb[:], spTb[:], maskt[:], ALU.add)
                        ptsb = wk.tile([PT, PT], BF16, tag="pts", bufs=3)
                        nc.scalar.activation(ptsb[:], spTb[:], AF.Exp)
                        nc.tensor.matmul(
                            ypB[:], ptsb[:], v_s[:, tB, :], start=False, stop=True,
                        )
                        for x0, ypX in ((a0, ypA), (b0, ypB)):
                            rinv = wk.tile([PT, 1], F32, tag="rinv", bufs=2)
                            nc.vector.reciprocal(rinv[:], ypX[:, HS:HS + 1])
                            y_sb = wk.tile([PT, HS], BF16, tag="y_sb", bufs=2)
                            nc.vector.tensor_scalar_mul(y_sb[:], ypX[:, 0:HS], rinv[:])
                            ty = psum.tile([PT, PT], BF16, tag="b", bufs=2)
                            nc.tensor.transpose(ty[:], y_sb[:], ident[:])
                            nc.vector.tensor_copy(yT_s[:, h, x0:x0 + PT], ty[:])
                    if proj_cb is not None:
                        proj_cb(mi)

            def _proj(ytf):
                # ---- output projection (column shard) ----
                for ti in range(NT):
                    t0 = ti * PT
                    pp = psum.tile([PT, QW], F32, tag="a", bufs=2)
                    for ci in range(NCT):
                        nc.tensor.matmul(
                            pp[:], ytf[:, ci, t0:t0 + PT], wp_s[:, ci, :],
                            start=(ci == 0), stop=(ci == NCT - 1),
                        )
                    ot = wk.tile([PT, QW], F32, tag="ot", bufs=2)
                    nc.vector.tensor_copy(ot[:], pp[:])
                    nc.sync.dma_start(out[t0:t0 + PT, :], ot[:])

            if loop_r is None:
                with tc.tile_pool(name="ytfp", bufs=1) as ytfp:
                    done = []

                    def proj_cb(mi):
                        # after pair mi: rows [0, (mi+1)*256) of y^T are final.
                        # chunk c needs pairs <= 2c+1; issue its AG then, and
                        # run its proj two pairs later (AG latency hidden).
                        for c in range(NCH):
                            if c == mi:
                                _ag_issue(c)
                        for c in range(NCH):
                            if c in done:
                                continue
                            if mi >= c + 2 or mi == NT // 2 - 1:
                                ytf = ytfp.tile([PT, NCT, TC], BF16, tag="ytf", bufs=2)
                                nc.sync.dma_start(
                                    ytf[:],
                                    ag_outs[c][:].rearrange("(c2 p) t -> p c2 t", p=PT),
                                )
                                _proj_chunk(c, ytf, local=True)
                                done.append(c)

                    with tc.tile_pool(name="xtp", bufs=1) as xtp:
                        xt_s = xtp.tile([PT, NCT, T], BF16)
                        _load_xt(xt_s)
                        _stage23(xt_s, proj_cb)
            else:
                # timing-only build: loop the whole compute body on-device;
                # proj consumes xt_s (same shape as gathered y^T) - numerics
                # are wrong but per-iteration work matches the real kernel
                # minus the AllGather.
                with tc.tile_pool(name="xtp", bufs=1) as xtp:
                    xt_s = xtp.tile([PT, NCT, T], BF16)

                    def proj_cb(mi):
                        _proj_chunk(mi, xt_s, local=False)

                    with tc.For_i(0, loop_r, 1):
                        _load_xt(xt_s)
                        _stage23(xt_s, proj_cb)

    nc.compile()
    return nc


def _tables(q_scale, k_scale):
    inv_freq = THETA ** (-np.arange(0, HS, 2, dtype=np.float64) / HS)
    ang = np.arange(T, dtype=np.float64)[:, None] * inv_freq[None, :]
    cosw = np.concatenate([np.cos(ang), np.cos(ang)], 1)  # (T, 128)
    sinw = np.concatenate([np.sin(ang), np.sin(ang)], 1)
    qs = np.asarray(q_scale, np.float64)
    ks = np.asarray(k_scale, np.float64)
    qs_rot = np.concatenate([qs[H2:], qs[:H2]])
    ks_rot = np.concatenate([ks[H2:], ks[:H2]])
    s = 1.0 / math.sqrt(HS)
    cosq = np.tile((cosw * qs[None, :] * s).astype(BF), (1, QH))
    sinq = np.tile((sinw * qs_rot[None, :] * s).astype(BF), (1, QH))
    cosk = (cosw * ks[None, :]).astype(BF)
    sink = (sinw * ks_rot[None, :]).astype(BF)
    return cosq, sinq, cosk, sink


def _make_in_maps(x, Wq, Wkv, Wproj, q_scale, k_scale, v1, value_lambda, layer_idx):
    x = np.asarray(x, np.float32)
    Wq = np.asarray(Wq, np.float32)
    Wkv = np.asarray(Wkv, np.float32)
    Wproj = np.asarray(Wproj, np.float32)

    li = int(np.asarray(layer_idx))
    mix = (v1 is not None) and (value_lambda is not None) and li > 0
    lam = float(np.asarray(value_lambda).reshape(())) if mix else 1.0

    cosq, sinq, cosk, sink = _tables(q_scale, k_scale)
    mneg = (np.tril(np.ones((PT, PT), np.float32), k=-1) * -1e30).astype(np.float32)

    in_maps = []
    for core in range(NCORES):
        b, r = core // TP, core % TP
        kcols = Wkv[:, r * HS:(r + 1) * HS]
        vcols = Wkv[:, NKV * HS + r * HS: NKV * HS + (r + 1) * HS]
        if mix:
            v1s_np = ((1.0 - lam) * np.asarray(v1, np.float32)[b, :, r, :]).astype(np.float32)
        else:
            v1s_np = np.zeros((T, HS), np.float32)
        in_maps.append({
            "xT": np.ascontiguousarray(x[b].T).astype(BF),
            "wq": Wq[:, r * QW:(r + 1) * QW].astype(BF),
            "wkv": np.ascontiguousarray(np.concatenate([kcols, vcols], 1)).astype(BF),
            "wp": np.ascontiguousarray(Wproj[:, r * QW:(r + 1) * QW]).astype(BF),
            "v1s": v1s_np,
            "cosq": cosq, "sinq": sinq, "cosk": cosk, "sink": sink,
            "mneg": mneg,
        })
    return in_maps


def kernel(x, Wq, Wkv, Wproj, q_scale, k_scale, v1, value_lambda, layer_idx):
    in_maps = _make_in_maps(x, Wq, Wkv, Wproj, q_scale, k_scale, v1,
                            value_lambda, layer_idx)
    if "nc" not in _CACHE:
        _CACHE["nc"] = _build()
    nc = _CACHE["nc"]

    trace = bool(int(os.environ.get("BASS_KERNEL_TRACE", "0")))
    res = run_bass_kernel_spmd(nc, in_maps, core_ids=list(range(NCORES)), trace=trace)
    _CACHE["last"] = res

    y = np.empty((B, T, C), np.float32)
    for core in range(NCORES):
        b, r = core // TP, core % TP
        y[b, :, r * QW:(r + 1) * QW] = np.asarray(res.results[core]["out"])
    return y


# revision 19
# speedup vs baseline: 1.3564x; 1.0077x over previous
"""GQA attention block (RMSNorm-QK + RoPE + causal attention + proj) on 8 TRN2 cores.

Sharding: DP=2 over batch x TP=4 over heads (4 q heads + 1 kv head per core).
Per core: x[b] @ Wq_shard / Wkv_shard -> q,k,v; RMSNorm+RoPE (cos/sin tables
precomputed on host, q_scale/k_scale and 1/sqrt(HS) baked in); causal
flash-ish attention in bf16 with f32 softmax stats; AllGather of y^T over the
4 TP ranks; column-parallel Wproj. Host pre-transposes x so the device never
transposes activations for the projections.
"""

import math
import os
import sys

import numpy as np

for _p in ("/opt/trn_rl_repo", "/root/.axon_site/_ro/trn_rl_repo"):
    if os.path.isdir(_p) and _p not in sys.path:
        sys.path.insert(0, _p)

import ml_dtypes

import concourse.bacc as bacc
import concourse.mybir as mybir
import concourse.tile as tile
from concourse import masks
from concourse.bass_utils import run_bass_kernel_spmd

BF16 = mybir.dt.bfloat16
F32 = mybir.dt.float32
AX = mybir.AxisListType
ALU = mybir.AluOpType
AF = mybir.ActivationFunctionType

B, T, C = 2, 2048, 2048
NH, NKV, HS = 16, 4, 128
TP = 4                # tensor-parallel ranks per batch element
QH = NH // TP         # q heads per core
QW = QH * HS          # 512
PT = 128
NT = T // PT          # 16
NCT = C // PT         # 16
H2 = HS // 2
EPS = 1e-6
THETA = 10000.0
NCORES = 8
BF = ml_dtypes.bfloat16

_CACHE = {}


def _build(loop_r=None):
    nc = bacc.Bacc(None, target_bir_lowering=False, num_devices=NCORES)

    xT = nc.declare_dram_parameter("xT", [C, T], BF16, isOutput=False)
    wq = nc.declare_dram_parameter("wq", [C, QW], BF16, isOutput=False)
    wkv = nc.declare_dram_parameter("wkv", [C, 2 * HS], BF16, isOutput=False)
    wp = nc.declare_dram_parameter("wp", [C, QW], BF16, isOutput=False)
    v1s = nc.declare_dram_parameter("v1s", [T, HS], F32, isOutput=False)
    cosq = nc.declare_dram_parameter("cosq", [T, QW], BF16, isOutput=False)
    sinq = nc.declare_dram_parameter("sinq", [T, QW], BF16, isOutput=False)
    cosk = nc.declare_dram_parameter("cosk", [T, HS], BF16, isOutput=False)
    sink = nc.declare_dram_parameter("sink", [T, HS], BF16, isOutput=False)
    mneg = nc.declare_dram_parameter("mneg", [PT, PT], F32, isOutput=False)
    out = nc.declare_dram_parameter("out", [T, QW], F32, isOutput=True)

    groups = [[0, 1, 2, 3], [4, 5, 6, 7]]

    with tile.TileContext(nc) as tc:
        with (
            tc.tile_pool(name="const", bufs=1) as const,
            tc.tile_pool(name="persist", bufs=1) as persist,
            tc.tile_pool(name="psum", bufs=1, space="PSUM") as psum,
            tc.tile_pool(name="wk", bufs=3) as wk,
            tc.tile_pool(name="dram", bufs=1, space="DRAM") as dram,
        ):
            ident = const.tile([PT, PT], BF16)
            masks.make_identity(nc, ident[:])
            maskt = const.tile([PT, PT], F32)
            nc.sync.dma_start(maskt[:], mneg[:])
            eps_t = const.tile([PT, 1], F32)
            nc.gpsimd.memset(eps_t[:], EPS)
            ones_t = const.tile([PT, 1], BF16)
            nc.gpsimd.memset(ones_t[:], 1.0)

            wq_s = persist.tile([PT, NCT, QW], BF16)
            wkv_s = persist.tile([PT, NCT, 2 * HS], BF16)
            wp_s = persist.tile([PT, NCT, QW], BF16)
            for ci in range(NCT):
                nc.sync.dma_start(wq_s[:, ci, :], wq[ci * PT:(ci + 1) * PT, :])
                nc.sync.dma_start(wkv_s[:, ci, :], wkv[ci * PT:(ci + 1) * PT, :])
                nc.sync.dma_start(wp_s[:, ci, :], wp[ci * PT:(ci + 1) * PT, :])

            qT_s = persist.tile([PT, QH, T], BF16)
            kT_s = persist.tile([PT, T], BF16)
            v_s = persist.tile([PT, NT, HS + 1], BF16)
            nc.gpsimd.memset(v_s[:, :, HS:HS + 1], 1.0)
            yT_s = persist.tile([PT, QH, T], BF16)
            q_all = persist.tile([PT, NT, QW], BF16)
            k_all = persist.tile([PT, NT, HS], BF16)
            ms_all = persist.tile([PT, NT, QH], F32)
            msk_all = persist.tile([PT, NT], F32)
            rs_all = persist.tile([PT, NT, QH], F32)
            rsk_all = persist.tile([PT, NT], F32)

            TC = 256
            NCH = T // TC
            ag_ins = [dram.tile([QW, TC], BF16, name=f"ag_in{c}") for c in range(NCH)]
            ag_outs = [dram.tile([C, TC], BF16, name=f"ag_out{c}") for c in range(NCH)]

            def _load_xt(xt_s):
                for ci in range(NCT):
                    nc.sync.dma_start(xt_s[:, ci, :], xT[ci * PT:(ci + 1) * PT, :])

            def _ag_issue(c):
                # gather chunk c of y^T across the TP group
                c0 = c * TC
                nc.sync.dma_start(
                    ag_ins[c][:].rearrange("(h p) t -> p h t", p=PT),
                    yT_s[:, :, c0:c0 + TC],
                )
                nc.gpsimd.collective_compute(
                    "AllGather", ALU.bypass, replica_groups=groups,
                    ins=[ag_ins[c][:]], outs=[ag_outs[c][:]],
                )

            def _proj_chunk(c, src, local):
                # project t-chunk c: 4 row-tiles of 128
                for tt in range(TC // PT):
                    ti = c * (TC // PT) + tt
                    lt0 = (tt if local else ti) * PT
                    pp = psum.tile([PT, QW], F32, tag="b", bufs=2)
                    for ci in range(NCT):
                        nc.tensor.matmul(
                            pp[:], src[:, ci, lt0:lt0 + PT], wp_s[:, ci, :],
                            start=(ci == 0), stop=(ci == NCT - 1),
                        )
                    ot = wk.tile([PT, QW], F32, tag="ot", bufs=2)
                    nc.vector.tensor_copy(ot[:], pp[:])
                    nc.sync.dma_start(out[ti * PT:(ti + 1) * PT, :], ot[:])

            def _stage23(xt_s, proj_cb=None):
                # Phase A: QKV projections + moment stats (ACT: Square only)
                for ti in range(NT):
                    t0 = ti * PT
                    qp = psum.tile([PT, QW], F32, tag="a", bufs=2)
                    for ci in range(NCT):
                        nc.tensor.matmul(
                            qp[:], xt_s[:, ci, t0:t0 + PT], wq_s[:, ci, :],
                            start=(ci == 0), stop=(ci == NCT - 1),
                        )
                    kvp = psum.tile([PT, 2 * HS], F32, tag="b", bufs=2)
                    for ci in range(NCT):
                        nc.tensor.matmul(
                            kvp[:], xt_s[:, ci, t0:t0 + PT], wkv_s[:, ci, :],
                            start=(ci == 0), stop=(ci == NCT - 1),
                        )
                    sq = wk.tile([PT, QW], F32, tag="sq", bufs=2)
                    nc.scalar.square(sq[:], qp[:])
                    nc.vector.tensor_reduce(
                        ms_all[:, ti, :], sq[:].rearrange("p (h d) -> p h d", d=HS),
                        AX.X, ALU.add,
                    )
                    sqk = wk.tile([PT, HS], F32, tag="sqk", bufs=2)
                    nc.scalar.square(sqk[:], kvp[:, 0:HS])
                    nc.vector.tensor_reduce(msk_all[:, ti:ti + 1], sqk[:], AX.X, ALU.add)
                    nc.vector.tensor_copy(q_all[:, ti, :], qp[:])
                    nc.vector.tensor_copy(k_all[:, ti, :], kvp[:, 0:HS])
                    v1t = wk.tile([PT, HS], F32, tag="v1t", bufs=2)
                    nc.sync.dma_start(v1t[:], v1s[t0:t0 + PT, :])
                    nc.vector.tensor_tensor(
                        v_s[:, ti, 0:HS], kvp[:, HS:2 * HS], v1t[:], ALU.add
                    )

                # Phase A2: batched rsqrt (ACT: Sqrt once)
                rs_f = rs_all[:].rearrange("p n h -> p (n h)")
                ms_f = ms_all[:].rearrange("p n h -> p (n h)")
                nc.scalar.activation(rs_f, ms_f, AF.Sqrt, bias=eps_t[:], scale=1.0 / HS)
                nc.vector.reciprocal(rs_f, rs_f)
                nc.scalar.activation(rsk_all[:], msk_all[:], AF.Sqrt, bias=eps_t[:], scale=1.0 / HS)
                nc.vector.reciprocal(rsk_all[:], rsk_all[:])

                # Phase B: normalize + RoPE + q/k transposes (no ACT)
                for ti in range(NT):
                    t0 = ti * PT
                    qn = wk.tile([PT, QH, HS], BF16, tag="qn", bufs=2)
                    for h in range(QH):
                        nc.vector.tensor_scalar_mul(
                            qn[:, h, :], q_all[:, ti, h * HS:(h + 1) * HS],
                            rs_all[:, ti, h:h + 1],
                        )
                    kn = wk.tile([PT, HS], BF16, tag="kn")
                    nc.vector.tensor_scalar_mul(kn[:], k_all[:, ti, :], rsk_all[:, ti:ti + 1])

                    qrot = wk.tile([PT, QH, HS], BF16, tag="qrot", bufs=2)
                    nc.vector.tensor_scalar_mul(qrot[:, :, 0:H2], qn[:, :, H2:HS], -1.0)
                    nc.vector.tensor_copy(qrot[:, :, H2:HS], qn[:, :, 0:H2])
                    krot = wk.tile([PT, HS], BF16, tag="krot")
                    nc.vector.tensor_scalar_mul(krot[:, 0:H2], kn[:, H2:HS], -1.0)
                    nc.vector.tensor_copy(krot[:, H2:HS], kn[:, 0:H2])

                    cqt = wk.tile([PT, QW], BF16, tag="cqt", bufs=2)
                    nc.sync.dma_start(cqt[:], cosq[t0:t0 + PT, :])
                    sqt = wk.tile([PT, QW], BF16, tag="sqt", bufs=2)
                    nc.sync.dma_start(sqt[:], sinq[t0:t0 + PT, :])
                    ckt = wk.tile([PT, HS], BF16, tag="ckt")
                    nc.sync.dma_start(ckt[:], cosk[t0:t0 + PT, :])
                    skt = wk.tile([PT, HS], BF16, tag="skt")
                    nc.sync.dma_start(skt[:], sink[t0:t0 + PT, :])

                    qn2 = qn[:].rearrange("p h d -> p (h d)")
                    qrot2 = qrot[:].rearrange("p h d -> p (h d)")
                    qr = wk.tile([PT, QW], BF16, tag="qr", bufs=2)
                    nc.vector.tensor_tensor(qr[:], qn2, cqt[:], ALU.mult)
                    nc.vector.tensor_tensor(qrot2, qrot2, sqt[:], ALU.mult)
                    nc.vector.tensor_tensor(qr[:], qr[:], qrot2, ALU.add)

                    kr = wk.tile([PT, HS], BF16, tag="kr")
                    nc.vector.tensor_tensor(kr[:], kn[:], ckt[:], ALU.mult)
                    nc.vector.tensor_tensor(krot[:], krot[:], skt[:], ALU.mult)
                    nc.vector.tensor_tensor(kr[:], kr[:], krot[:], ALU.add)

                    qr3 = qr[:].rearrange("p (h d) -> p h d", d=HS)
                    for h in range(QH):
                        tq = psum.tile([PT, PT], BF16, tag="c", bufs=2)
                        nc.tensor.transpose(tq[:], qr3[:, h, :], ident[:])
                        nc.vector.tensor_copy(qT_s[:, h, t0:t0 + PT], tq[:])
                    tk = psum.tile([PT, PT], BF16, tag="c", bufs=2)
                    nc.tensor.transpose(tk[:], kr[:], ident[:])
                    nc.vector.tensor_copy(kT_s[:, t0:t0 + PT], tk[:])

                # Phase C: causal attention, row QUADS (ACT: Exp only).
                # s^T = k^T(stat) . q^T(mov, 512 wide); exp -> p^T in SBUF;
                # y+rowsum from one matmul vs v_aug (ones column).
                YP_TAGS = ("c", "d", "c", "d")
                for qi in range(NT // 4):
                    t0r = 4 * qi           # first row-tile of the quad
                    q0 = t0r * PT          # col offset of quad in qT
                    yps = [psum.tile([PT, HS + 1], F32, tag=YP_TAGS[r], bufs=2,
                                     name=f"yp{qi}_{r}")
                           for r in range(4)]
                    for h in range(QH):
                        if h > 0:
                            yps = [psum.tile([PT, HS + 1], F32, tag=YP_TAGS[r],
                                             bufs=2, name=f"yp{qi}_{h}_{r}")
                                   for r in range(4)]
                        for j in range(t0r):
                            spT = psum.tile([PT, 4 * PT], F32, tag="a", bufs=2)
                            nc.tensor.matmul(
                                spT[:], kT_s[:, j * PT:(j + 1) * PT],
                                qT_s[:, h, q0:q0 + 4 * PT], start=True, stop=True,
                            )
                            pts = wk.tile([PT, 4 * PT], BF16, tag="pts", bufs=2)
                            nc.scalar.activation(pts[:], spT[:], AF.Exp)
                            for r in range(4):
                                nc.tensor.matmul(
                                    yps[r][:], pts[:, r * PT:(r + 1) * PT],
                                    v_s[:, j, :],
                                    start=(j == 0), stop=False,
                                )
                        # ragged diagonal block: j = t0r+d covers rows d..3
                        for d in range(4):
                            j = t0r + d
                            w = (4 - d) * PT
                            spT = psum.tile([PT, 4 * PT], F32, tag="a", bufs=2)
                            nc.tensor.matmul(
                                spT[:, 0:w], kT_s[:, j * PT:(j + 1) * PT],
                                qT_s[:, h, j * PT:q0 + 4 * PT],
                                start=True, stop=True,
                            )
                            nc.vector.tensor_tensor(
                                spT[:, 0:PT], spT[:, 0:PT], maskt[:], ALU.add
                            )
                            pts = wk.tile([PT, 4 * PT], BF16, tag="pts", bufs=2)
                            nc.scalar.activation(pts[:, 0:w], spT[:, 0:w], AF.Exp)
                            for idx, r in enumerate(range(d, 4)):
                                nc.tensor.matmul(
                                    yps[r][:], pts[:, idx * PT:(idx + 1) * PT],
                                    v_s[:, j, :],
                                    start=(j == 0), stop=(d == r),
                                )
                        for r in range(4):
                            x0 = (t0r + r) * PT
                            rinv = wk.tile([PT, 1], F32, tag="rinv", bufs=2)
                            nc.vector.reciprocal(rinv[:], yps[r][:, HS:HS + 1])
                            y_sb = wk.tile([PT, HS], BF16, tag="y_sb", bufs=2)
                            nc.vector.tensor_scalar_mul(y_sb[:], yps[r][:, 0:HS], rinv[:])
                            ty = psum.tile([PT, PT], BF16, tag="b", bufs=2)
                            nc.tensor.transpose(ty[:], y_sb[:], ident[:])
                            nc.vector.tensor_copy(yT_s[:, h, x0:x0 + PT], ty[:])
                    if proj_cb is not None:
                        proj_cb(qi)

            if loop_r is None:
                with tc.tile_pool(name="ytfp", bufs=1) as ytfp:
                    done = []

                    def proj_cb(qi):
                        # after quad qi: rows [0, (qi+1)*512) of y^T are final
                        # -> chunks 2qi, 2qi+1 can gather; proj runs one quad
                        # later so the AllGather latency hides behind compute.
                        for c in (2 * qi, 2 * qi + 1):
                            _ag_issue(c)
                        for c in range(NCH):
                            if c in done:
                                continue
                            if c <= 2 * qi - 1 or qi == NT // 4 - 1:
                                ytf = ytfp.tile([PT, NCT, TC], BF16, tag="ytf", bufs=2)
                                nc.sync.dma_start(
                                    ytf[:],
                                    ag_outs[c][:].rearrange("(c2 p) t -> p c2 t", p=PT),
                                )
                                _proj_chunk(c, ytf, local=True)
                                done.append(c)

                    with tc.tile_pool(name="xtp", bufs=1) as xtp:
                        xt_s = xtp.tile([PT, NCT, T], BF16)
                        _load_xt(xt_s)
                        _stage23(xt_s, proj_cb)
            else:
                # timing-only build: loop the whole compute body on-device;
                # proj consumes xt_s (same shape as gathered y^T) - numerics
                # are wrong but per-iteration work matches the real kernel
                # minus the AllGather.
                with tc.tile_pool(name="xtp", bufs=1) as xtp:
                    xt_s = xtp.tile([PT, NCT, T], BF16)

                    def proj_cb(qi):
                        _proj_chunk(2 * qi, xt_s, local=False)
                        _proj_chunk(2 * qi + 1, xt_s, local=False)

                    with tc.For_i(0, loop_r, 1):
                        _load_xt(xt_s)
                        _stage23(xt_s, proj_cb)

    nc.compile()
    return nc


def _tables(q_scale, k_scale):
    inv_freq = THETA ** (-np.arange(0, HS, 2, dtype=np.float64) / HS)
    ang = np.arange(T, dtype=np.float64)[:, None] * inv_freq[None, :]
    cosw = np.concatenate([np.cos(ang), np.cos(ang)], 1)  # (T, 128)
    sinw = np.concatenate([np.sin(ang), np.sin(ang)], 1)
    qs = np.asarray(q_scale, np.float64)
    ks = np.asarray(k_scale, np.float64)
    qs_rot = np.concatenate([qs[H2:], qs[:H2]])
    ks_rot = np.concatenate([ks[H2:], ks[:H2]])
    s = 1.0 / math.sqrt(HS)
    cosq = np.tile((cosw * qs[None, :] * s).astype(BF), (1, QH))
    sinq = np.tile((sinw * qs_rot[None, :] * s).astype(BF), (1, QH))
    cosk = (cosw * ks[None, :]).astype(BF)
    sink = (sinw * ks_rot[None, :]).astype(BF)
    return cosq, sinq, cosk, sink


def _make_in_maps(x, Wq, Wkv, Wproj, q_scale, k_scale, v1, value_lambda, layer_idx):
    x = np.asarray(x, np.float32)
    Wq = np.asarray(Wq, np.float32)
    Wkv = np.asarray(Wkv, np.float32)
    Wproj = np.asarray(Wproj, np.float32)

    li = int(np.asarray(layer_idx))
    mix = (v1 is not None) and (value_lambda is not None) and li > 0
    lam = float(np.asarray(value_lambda).reshape(())) if mix else 1.0

    cosq, sinq, cosk, sink = _tables(q_scale, k_scale)
    mneg = (np.tril(np.ones((PT, PT), np.float32), k=-1) * -1e30).astype(np.float32)

    in_maps = []
    for core in range(NCORES):
        b, r = core // TP, core % TP
        kcols = Wkv[:, r * HS:(r + 1) * HS]
        vcols = Wkv[:, NKV * HS + r * HS: NKV * HS + (r + 1) * HS]
        if mix:
            v1s_np = ((1.0 - lam) * np.asarray(v1, np.float32)[b, :, r, :]).astype(np.float32)
        else:
            v1s_np = np.zeros((T, HS), np.float32)
        in_maps.append({
            "xT": np.ascontiguousarray(x[b].T).astype(BF),
            "wq": Wq[:, r * QW:(r + 1) * QW].astype(BF),
            "wkv": np.ascontiguousarray(np.concatenate([kcols, vcols], 1)).astype(BF),
            "wp": np.ascontiguousarray(Wproj[:, r * QW:(r + 1) * QW]).astype(BF),
            "v1s": v1s_np,
            "cosq": cosq, "sinq": sinq, "cosk": cosk, "sink": sink,
            "mneg": mneg,
        })
    return in_maps


def kernel(x, Wq, Wkv, Wproj, q_scale, k_scale, v1, value_lambda, layer_idx):
    in_maps = _make_in_maps(x, Wq, Wkv, Wproj, q_scale, k_scale, v1,
                            value_lambda, layer_idx)
    if "nc" not in _CACHE:
        _CACHE["nc"] = _build()
    nc = _CACHE["nc"]

    trace = bool(int(os.environ.get("BASS_KERNEL_TRACE", "0")))
    res = run_bass_kernel_spmd(nc, in_maps, core_ids=list(range(NCORES)), trace=trace)
    _CACHE["last"] = res

    y = np.empty((B, T, C), np.float32)
    for core in range(NCORES):
        b, r = core // TP, core % TP
        y[b, :, r * QW:(r + 1) * QW] = np.asarray(res.results[core]["out"])
    return y
